# revision 49
# baseline (speedup 1.0000x reference)
"""Trainium2 Bass kernel for nn_MoEDiscriminator (8 experts, MLP 64->256->256->1).

Strategy (data-parallel over 8 NeuronCores):
- st [65536, 64] is sharded along batch: 8192 rows per core; expert weights
  are replicated on every core.
- All matmul operands are bf16 (host-converted); PSUM accumulates fp32.
  Measured end-to-end error vs the fp32 reference is ~4e-3 relative.
- Activations live as [feature_on_partitions, batch_on_free] SBUF tiles.
- Per expert c, per batch tile of 512:
    L1: K=64, so the two hidden halves run as a row-tiled PAIR: half 0 at
        PE rows 0-63, half 1 at rows 64-127 (st is duplicated across the
        two row groups). The two matmuls execute concurrently -> 2x L1.
    L2: 4 full 128x128 matmuls (2 out-halves x 2 k-chunks).
    L3: M=32 zero-padded stationaries place expert c at PSUM partition
        32*(c%4) + 16*(c//4) via tile_position col groups; all 8 experts
        accumulate into ONE PSUM bank per tile (group-sequential issue
        order measured faster than round-robin). One [113, 512] copy
        evicts it; 8 single-row DMAs write d at the end.
- relu(x + b) evictions run on ScalarE/VectorE, load-balanced; b3 is added
  on the host; output is reassembled host-side to [65536, 8, 1].
- Tuning (HW-measured): PSUM bufs (4,3,1) >> (3,3,2)/(3,4,1); tight
  H1/H2 pools beat larger ones; wider paired evictions and interleaved
  tile-pair schedules all measured SLOWER despite modeling better.
- Operand ADJACENCY matters most: all of an expert's weights live in one
  contiguous mega-pack (W1 | W2 in (j,k) issue order | W3) so the
  LDWEIGHTS stream walks monotonically through SBUF, and each item's
  h1/h2 halves share one [128, 1024] SBUF tile (ACT writes cols 0:512,
  DVE 512:1024) so consecutive matmuls stream adjacent moving operands
  with per-half dependencies. Each adjacency step measured faster
  (~195 -> 165 -> ~179-197us draws; full-run 207 -> 179us).
"""

import sys

sys.path.insert(0, "/opt/trn_rl_repo")
from contextlib import ExitStack

import numpy as np
import ml_dtypes

import concourse.bass as bass
import concourse.tile as tile
from concourse import bacc, mybir
from concourse.bass import ts
from concourse.bass_utils import run_bass_kernel_spmd

P = 128
C = 8            # experts
DS = 64          # input feature dim
H = 256          # hidden width
B = 65536        # full batch
NCORES = 8
NB = B // NCORES  # 8192 rows per core
BT = 512         # batch tile (free dim of matmuls)
NT = NB // BT    # 16
ST_CHUNKS = (512, 1536, 2048, 4096)   # graduated st chunk widths
PSUM_BUFS = (4, 3, 1)                 # (psumA, psumB, psumD)
H1_BUFS = 2
H2_BUFS = 5
ACT_EXTRA = 0     # if > 0: every ACT_EXTRA-th item, ACT also takes L2 h1 evict

f32 = mybir.dt.float32
bf16 = mybir.dt.bfloat16
AF = mybir.ActivationFunctionType
ALU = mybir.AluOpType
BF_NP = ml_dtypes.bfloat16

_NC_CACHE = {}


def _build_nc(repeats=1):
    key = (repeats, ST_CHUNKS, PSUM_BUFS, H1_BUFS, H2_BUFS, ACT_EXTRA)
    if key in _NC_CACHE:
        return _NC_CACHE[key]
    nc = bacc.Bacc("TRN2", target_bir_lowering=False, debug=False,
                   num_devices=NCORES)
    st_d = nc.dram_tensor("st", [P, NB], bf16, kind="ExternalInput").ap()
    bias_d = nc.dram_tensor("bias", [P, 4 * C], f32, kind="ExternalInput").ap()
    # mw: per-expert weight mega-pack, contiguous in LDWEIGHTS issue order:
    #   cols   0:128  W1 pack (partitions 0-63 = half 0, 64-127 = half 1)
    #   cols 128:640  W2 chunks in (j, k) order: col 128 + 128*(2j+k)
    #   cols 640:704  W3 32-wide zero-padded stationaries, col 640 + 32k;
    #                 expert c sits in stationary column 16*(c//4), landing on
    #                 PSUM partition 32*(c%4) + 16*(c//4) so one copy evicts
    #                 all 8 experts of a tile from a single bank.
    MWC = 704
    mw_d = nc.dram_tensor("mw", [C, P, MWC], bf16, kind="ExternalInput").ap()
    d_d = nc.dram_tensor("d", [C, NB], f32, kind="ExternalOutput").ap()

    with tile.TileContext(nc) as tc, ExitStack() as ctx:
        const = ctx.enter_context(tc.tile_pool(name="const", bufs=2))
        work1 = ctx.enter_context(tc.tile_pool(name="work1", bufs=H1_BUFS))
        work2 = ctx.enter_context(tc.tile_pool(name="work2", bufs=H2_BUFS))
        psumA = ctx.enter_context(
            tc.tile_pool(name="psumA", bufs=PSUM_BUFS[0], space="PSUM"))
        psumB = ctx.enter_context(
            tc.tile_pool(name="psumB", bufs=PSUM_BUFS[1], space="PSUM"))
        psumD = ctx.enter_context(
            tc.tile_pool(name="psumD", bufs=PSUM_BUFS[2], space="PSUM"))

        def body():
            # Bias table first (tiny), then expert-0's mega-pack so the first
            # L1 matmul waits on a single small transfer.
            bias_sb = const.tile([P, 4 * C], f32)
            nc.sync.dma_start(bias_sb[:], bias_d)
            b1_sb = bias_sb[:, 0:2 * C]
            b2_sb = bias_sb[:, 2 * C:4 * C]

            # One st tile, DMA'd in graduated column slices: first L1 waits
            # only on the small first slice, but every batch tile then streams
            # from adjacent addresses of a single SBUF tensor.
            st_sb = const.tile([P, NB], bf16, name="st_sb")
            mw_sb = const.tile([P, C, MWC], bf16)
            d_sb = const.tile([P, NB], f32)  # experts live on partitions 16*r
            # Per-expert weight chunks interleaved with st so mw[c] lands
            # before expert c's L1/L2 reach it in the pipeline.
            nc.sync.dma_start(mw_sb[:, 0:1],
                              mw_d[0:1].rearrange("c p f -> p c f"))
            nc.sync.dma_start(st_sb[:, 0:ST_CHUNKS[0]], st_d[:, 0:ST_CHUNKS[0]])
            nc.sync.dma_start(mw_sb[:, 1:2],
                              mw_d[1:2].rearrange("c p f -> p c f"))
            nc.sync.dma_start(mw_sb[:, 2:4],
                              mw_d[2:4].rearrange("c p f -> p c f"))
            off = ST_CHUNKS[0]
            nc.sync.dma_start(st_sb[:, off:off + ST_CHUNKS[1]],
                              st_d[:, off:off + ST_CHUNKS[1]])
            nc.sync.dma_start(mw_sb[:, 4:6],
                              mw_d[4:6].rearrange("c p f -> p c f"))
            nc.sync.dma_start(mw_sb[:, 6:8],
                              mw_d[6:8].rearrange("c p f -> p c f"))
            off += ST_CHUNKS[1]
            for i in range(2, len(ST_CHUNKS)):
                nc.sync.dma_start(st_sb[:, off:off + ST_CHUNKS[i]],
                                  st_d[:, off:off + ST_CHUNKS[i]])
                off += ST_CHUNKS[i]
            assert off == NB

            def st_slice(t, r):
                return st_sb[64 * r:64 * r + 64, ts(t, BT)]

            def w1_ap(c, r):
                return mw_sb[64 * r:64 * r + 64, c, 0:P]

            h1s, h2s = {}, {}

            def stage_l1(i):
                t, c = divmod(i, C)
                pA = [psumA.tile([P, BT], f32, tag="pA", name=f"pA{r}")
                      for r in range(2)]
                for r in range(2):
                    nc.tensor.matmul(pA[r][:], w1_ap(c, r), st_slice(t, r),
                                     start=True, stop=True)
                h1 = work1.tile([P, 2 * BT], bf16, tag="h1", name="h1")
                nc.scalar.activation(h1[:, 0:BT], pA[0][:], AF.Relu,
                                     bias=b1_sb[:, 2 * c:2 * c + 1])
                nc.vector.tensor_scalar(h1[:, BT:2 * BT], pA[1][:],
                                        b1_sb[:, 2 * c + 1:2 * c + 2],
                                        0.0, ALU.add, ALU.max)
                h1s[i] = h1

            def stage_l2(i):
                t, c = divmod(i, C)
                h1 = h1s.pop(i)
                pB = [psumB.tile([P, BT], f32, tag="pB", name=f"pB{j}")
                      for j in range(2)]
                for j in range(2):
                    for k in range(2):
                        o = P + P * (2 * j + k)
                        nc.tensor.matmul(pB[j][:], mw_sb[:, c, o:o + P],
                                         h1[:, k * BT:(k + 1) * BT],
                                         start=(k == 0), stop=(k == 1))
                h2 = work2.tile([P, 2 * BT], bf16, tag="h2", name="h2")
                nc.scalar.activation(h2[:, 0:BT], pB[0][:], AF.Relu,
                                     bias=b2_sb[:, 2 * c:2 * c + 1])
                nc.vector.tensor_scalar(h2[:, BT:2 * BT], pB[1][:],
                                        b2_sb[:, 2 * c + 1:2 * c + 2],
                                        0.0, ALU.add, ALU.max)
                h2s[i] = h2

            pDs = {}

            def stage_l3_quad(m):
                # m = item index of the quad's last expert (c % 4 == 3)
                t, c_last = divmod(m, C)
                q = c_last // 4
                if q == 0:
                    pDs[t] = psumD.tile([P, BT], f32, tag="pD", name="pD")
                pD = pDs[t]
                for j in range(4):
                    for k in range(2):
                        e = 4 * q + j
                        h2 = h2s[t * C + e]
                        o = 5 * P + 32 * k
                        nc.tensor.matmul(pD[32 * j:32 * j + 32, :],
                                         mw_sb[:, e, o:o + 32],
                                         h2[:, k * BT:(k + 1) * BT],
                                         start=(q == 0 and k == 0),
                                         stop=(q == 1 and k == 1),
                                         tile_position=(0, 32 * j),
                                         skip_group_check=True)
                for j in range(4):
                    h2s.pop(t * C + 4 * q + j)
                if q == 1:
                    pD = pDs.pop(t)
                    if t % 2 == 0:
                        nc.scalar.copy(d_sb[0:113, ts(t, BT)], pD[0:113, :])
                    else:
                        nc.vector.tensor_copy(d_sb[0:113, ts(t, BT)],
                                              pD[0:113, :])

            N = NT * C
            for i in range(N):
                stage_l1(i)
                if i >= 1:
                    stage_l2(i - 1)
                m = i - 2
                if m >= 0 and m % 4 == 3:
                    stage_l3_quad(m)
            stage_l2(N - 1)
            stage_l3_quad(N - 1)
            for r in range(C):
                nc.sync.dma_start(d_d[r:r + 1, :], d_sb[16 * r:16 * r + 1, :])

        for _rep in range(repeats):
            body()

    nc.compile()
    _NC_CACHE[key] = nc
    return nc


def _prep_weights(W1, b1, W2, b2, W3):
    # Mega-pack: see mw_d comment in _build_nc.
    MW = np.zeros((C, P, 704), BF_NP)
    for c in range(C):
        MW[c, 0:DS, 0:P] = W1[c][:, 0:P].astype(BF_NP)
        MW[c, DS:P, 0:P] = W1[c][:, P:2 * P].astype(BF_NP)
        for j in range(2):
            for k in range(2):
                o = P + P * (2 * j + k)
                MW[c, :, o:o + P] = W2[c][k * P:(k + 1) * P,
                                         j * P:(j + 1) * P].astype(BF_NP)
        m = 16 * (c // 4)
        for k in range(2):
            MW[c, :, 5 * P + 32 * k + m] = W3[c, k * P:(k + 1) * P, 0].astype(BF_NP)
    b1h = np.ascontiguousarray(b1.reshape(C * 2, P).T)
    b2h = np.ascontiguousarray(b2.reshape(C * 2, P).T)
    bias = np.concatenate([b1h, b2h], axis=1).astype(np.float32)  # [128, 32]
    return MW, bias


def _make_in_maps(st, W1, b1, W2, b2, W3):
    MW, bias = _prep_weights(W1, b1, W2, b2, W3)
    in_maps = []
    for core in range(NCORES):
        shard = st[core * NB:(core + 1) * NB]            # [8192, 64]
        stT = np.ascontiguousarray(
            np.concatenate([shard.T, shard.T], axis=0).astype(BF_NP))
        in_maps.append({"st": stT, "bias": bias, "mw": MW})
    return in_maps


class _SpmdExec:
    """Reusable jitted shard_map executor for a compiled Bass module
    (mirrors concourse.bass2jax.run_bass_via_pjrt; verified bit-identical)."""

    def __init__(self, nc, n_cores):
        import jax
        from jax.sharding import Mesh, PartitionSpec
        from jax.experimental.shard_map import shard_map
        from concourse.bass2jax import (_bass_exec_p, partition_id_tensor,
                                        install_neuronx_cc_hook)

        install_neuronx_cc_hook()
        self.n_cores = n_cores
        in_names, out_names, out_avals = [], [], []
        pname = nc.partition_id_tensor.name if nc.partition_id_tensor else None
        for alloc in nc.m.functions[0].allocations:
            if not isinstance(alloc, mybir.MemoryLocationSet):
                continue
            name = alloc.memorylocations[0].name
            if alloc.kind == "ExternalInput":
                if name != pname:
                    in_names.append(name)
            elif alloc.kind == "ExternalOutput":
                out_names.append(name)
                out_avals.append(jax.core.ShapedArray(
                    tuple(alloc.tensor_shape), mybir.dt.np(alloc.dtype)))
        self.in_names, self.out_names, self.out_avals = \
            in_names, out_names, out_avals
        all_in = in_names + out_names + ([pname] if pname else [])

        def _bdy(*args):
            ops = list(args)
            if pname is not None:
                ops.append(partition_id_tensor())
            return tuple(_bass_exec_p.bind(
                *ops, out_avals=tuple(out_avals), in_names=tuple(all_in),
                out_names=tuple(out_names), lowering_input_output_aliases=(),
                sim_require_finite=True, sim_require_nnan=True, nc=nc))

        mesh = Mesh(np.asarray(jax.devices()[:n_cores]), ("core",))
        nio = len(in_names) + len(out_names)
        self.sharded = jax.jit(
            shard_map(_bdy, mesh=mesh,
                      in_specs=(PartitionSpec("core"),) * nio,
                      out_specs=(PartitionSpec("core"),) * len(out_names),
                      check_rep=False),
            keep_unused=True)

    def run(self, in_maps):
        args = [np.concatenate([np.asarray(m[n]) for m in in_maps], axis=0)
                for n in self.in_names]
        args += [np.zeros((self.n_cores * a.shape[0], *a.shape[1:]), a.dtype)
                 for a in self.out_avals]
        outs = self.sharded(*args)
        return [{n: np.asarray(outs[i]).reshape(
                    self.n_cores, *self.out_avals[i].shape)[c]
                 for i, n in enumerate(self.out_names)}
                for c in range(self.n_cores)]


_EXEC_CACHE = {}


def _run_spmd(nc, in_maps, first_call):
    """First call goes through bass_utils.run_bass_kernel_spmd; later calls
    reuse a cached PJRT executable (bit-identical output, no re-jit)."""
    if not first_call:
        ex = _EXEC_CACHE.get(id(nc))
        if ex is None:
            ex = _EXEC_CACHE[id(nc)] = _SpmdExec(nc, NCORES)
        return ex.run(in_maps)
    import os
    try:
        return run_bass_kernel_spmd(
            nc, in_maps, core_ids=list(range(NCORES))).results
    except ModuleNotFoundError:
        # BASS_TRACE set but the axon NTFF hook module is absent: force
        # trace off and retry.
        os.environ["BASS_NEVER_TRACE"] = "1"
        return run_bass_kernel_spmd(
            nc, in_maps, core_ids=list(range(NCORES))).results


_CALLED = False


def kernel(st, W1, b1, W2, b2, W3, b3):
    global _CALLED
    st = np.ascontiguousarray(np.asarray(st, np.float32))
    in_maps = _make_in_maps(
        st,
        np.asarray(W1, np.float32), np.asarray(b1, np.float32),
        np.asarray(W2, np.float32), np.asarray(b2, np.float32),
        np.asarray(W3, np.float32))
    nc = _build_nc(1)
    results = _run_spmd(nc, in_maps, first_call=not _CALLED)
    _CALLED = True

    b3v = np.asarray(b3, np.float32).reshape(1, C)
    # d row r holds expert 4 * (r % 2) + r // 2 (PSUM partition 16 * r maps
    # to col group r // 2, quad r % 2).
    perm = [4 * (r % 2) + r // 2 for r in range(C)]
    out = np.empty((B, C, 1), np.float32)
    for core in range(NCORES):
        d = results[core]["d"]                            # [8, 8192]
        out[core * NB:(core + 1) * NB, :, 0][:, perm] = d.T
    out[:, :, 0] += b3v
    return out
